# revision 66
# baseline (speedup 1.0000x reference)
"""Causal multi-head attention (B=4, S=2048, D=1024, H=16) on 8 TRN2 cores.

Sharding (DP on batch x TP on heads): core 2b+g handles batch b and heads
8g..8g+8.  Each core computes the qkv projection for its head group, causal
attention, and a partial output projection (its heads' rows of w_proj, with
b_proj/2 folded in); the host sums the two fp16 partials per batch -- no
device collectives.

Device kernel design (single SPMD program, Tile framework):
- No on-device transpose of inputs: host sends x[b] transposed; q/k are
  produced transposed ([douts, rows], head-pair packed: even head on
  partitions 0:64, odd on 64:128), v natural with a ones column per head.
- Scores are computed transposed ([keys, q]).  Block-causal: fully-masked
  key tiles are skipped, fully-masked column ranges of diagonal tiles are
  skipped in the matmul, and the remaining 128-wide triangle is zeroed on
  the exp'd probs with a precomputed bf16 mask (DVE 2x).  exp runs without
  max-subtraction (scores are small; validated 2e-6 vs reference).
- pv runs in the "a-natural" orientation: psum tile [128 queries, 65] with
  the probs block as the stationary operand and the v(+ones) block moving.
  The tensor engine charges by moving-dim size, so this streams 65 columns
  per (query-block, key-tile) instead of ~512 per key-tile -- half the PE
  time of the aT-producing orientation the baseline used.  The ones column
  lands the softmax denominator in psum column 64, per-query on partitions,
  so normalization is one DVE reciprocal + tensor_scalar multiply per
  query-block pair; no partition broadcast is needed.
- a [q, dm] becomes aT [dm, q] for the out-projection via the DMA xbar
  transpose (16x128 tiles, fp16) on the sync queue for head-pairs 0-2
  (latency hidden one window later), and via PE transpose (identity
  matmul) + DVE copy during hp3, where the transpose gates the
  out-projection and the ~2.4us DMA issue+sem latency would stall PE.
- Schedule: software-pipelined one chunk deep.  Window (hp, c) emits score
  pairs (diagonal pairs first) and their exps; between pairs it drains the
  previous chunk's pv chains/normalize (slot gi==1) and slices of the next
  head-pair's qkv projection -- or out-projection row-groups during hp3 --
  so the in-order PE queue always has ready work while ACT churns exp (ACT
  is the second-busiest engine at ~159us).  qkv chunks n>=2 of head-pair 3
  are deferred into hp3's early filler slots.  The hp0 ramp runs all eight
  q/k psum groups kt-major (two groups share each 2-bank scores slot) so PE
  consumes input tiles in DMA-arrival order; input DMAs put wq on sync and
  wk on scalar (the two HWDGE queues), the early x tiles on the gpsimd
  SWDGE queue (no HWDGE contention; the gpsimd-built masks throttle the
  big x transfers so the small w tiles win the shared DMA device early),
  wv last; the hp0 v-projection's last two groups ride the filler queue.  A few dep-free warmup matmuls on a
  memset scratch tile cover the first-DMA latency and pre-ramp the PE
  clock.
- PSUM: exactly 8 banks -- scores 2x[128,1024] double-buffered (4), pv
  accumulators 2x[128,260] (2, also hosting the hp3 transpose tiles),
  qkv/out-projection [128,512] x2 (2); the hp0 ramp borrows the idle
  scores/pv slots.

Precision: fp16 matmul operands (11-bit mantissa, ~= tf32), fp32 PSUM
accumulation, probs in bf16 (needs fp32-range exponent), fp16 output
partials summed in fp32 on host.  Measured 1.813e-3 max relative error
vs the fp32 reference on hardware.

TimelineSim cost model: 228,031 ns per core (baseline kernel: 261,568).
PE busy ~202us (the fp16 floor for this decomposition is ~197us: qk proj
131k + v proj 66k + scores 139k (K=64-bound) + pv 71k + out-proj 66k
cycles at 2.4GHz); residual idle is the DMA-bandwidth-bound startup ramp
(~8.4MB of inputs), ACT-paced score windows, and the drain tail.
"""

from collections import deque

import numpy as np

import concourse.bass as bass
import concourse.mybir as mybir
from concourse import bacc
from concourse.bass import ds
from concourse.tile import TileContext

F16 = mybir.dt.float16
F32 = mybir.dt.float32
BF16 = mybir.dt.bfloat16

S = 2048  # sequence length
D = 1024  # model dim
HD = 64  # head dim
HPC = 8  # heads per core
GD = HPC * HD  # 512, per-core qkv width
N_CORES = 8

AF = mybir.ActivationFunctionType
ALU = mybir.AluOpType

# hp0 qkv accumulation order, matched to input-tile DMA arrival
KT_ORDER = list(range(8))


def build_bass(nloop=1):
    nc = bacc.Bacc(None, target_bir_lowering=False)

    xT_d = nc.dram_tensor("xT", [D, S], F16, kind="ExternalInput")
    wq_d = nc.dram_tensor("wq", [D, GD], F16, kind="ExternalInput")
    wk_d = nc.dram_tensor("wk", [D, GD], F16, kind="ExternalInput")
    wv_d = nc.dram_tensor("wv", [D, GD], F16, kind="ExternalInput")
    wp_d = nc.dram_tensor("wp", [GD, D], F16, kind="ExternalInput")
    bq_d = nc.dram_tensor("bq", [128, 4], F32, kind="ExternalInput")
    bk_d = nc.dram_tensor("bk", [128, 4], F32, kind="ExternalInput")
    bv_d = nc.dram_tensor("bv", [128, GD], F32, kind="ExternalInput")
    bp_d = nc.dram_tensor("bp", [128, D], F32, kind="ExternalInput")
    id_d = nc.dram_tensor("ident", [128, 128], F16, kind="ExternalInput")
    out_d = nc.dram_tensor("out", [S, D], F16, kind="ExternalOutput")

    with TileContext(nc) as tc:
     for _loop in range(nloop):
      with tc.tile_pool(name="persist", bufs=1) as persist:
        # Per-head-pair q/k (transposed [douts, rows]; partitions 0:64 =
        # even head dims, 64:128 = odd head dims) and v (natural [keys,
        # per-pair 2*65] with a ones column per head at local col 64 so the
        # pv matmul also emits the softmax denominator as column 64).
        qTs, kTs, vs = [], [], []
        for hp in range(4):
            qrow, krow = [], []
            for n in range(4):
                t_q = persist.tile([128, 512], F16, tag=f"qT{hp}_{n}")
                t_k = persist.tile([128, 512], F16, tag=f"kT{hp}_{n}")
                qrow.append(t_q)
                krow.append(t_k)
            vrow = []
            for g in range(4):
                t_v = persist.tile([128, 4 * 130], BF16, tag=f"v{hp}_{g}")
                vrow.append(t_v)
            qTs.append(qrow)
            kTs.append(krow)
            vs.append(vrow)
        bq_sb = persist.tile([128, 4], F32)
        bk_sb = persist.tile([128, 4], F32)
        bv_sb = persist.tile([128, GD], F32)
        wp_sb = persist.tile([128, 4 * D], F16)
        bp_sb = persist.tile([128, D], F32)

        # aT per-chunk tiles: aTc[c] = [128, 4*512], columns hp-major
        # (hp*512 + q-within-chunk); partitions = head-pair dm packing
        aTc = []
        for c in range(4):
            aTc_t = persist.tile([128, 4 * 512], F16, tag=f"aTc{c}")
            aTc.append(aTc_t)

        # Precomputed causal mask tiles, packed: for diagonal offset
        # d = j*128 only columns [d:512) are ever used, and in that sliced
        # frame the triangle is always mask[i, qq] = 1 if qq >= i else 0.
        MOFF = [0, 512, 896, 1152]  # packed offsets, widths 512-128j
        ident = persist.tile([128, 128], F16)
        # PE warmup scratch: dep-free matmuls fill the first-input-DMA
        # latency and pre-ramp the tensor-engine clock
        dums = persist.tile([128, 512], BF16)
        nc.vector.memset(dums[:, :], 1.0)
        # built on gpsimd before the SWDGE x loads: the ~3us of Pool-queue
        # work throttles the big x transfers so the small wq/wk tiles (the
        # kt-major ramp's first needs) win the shared DMA device early
        masks = persist.tile([128, 1280], BF16)
        nc.gpsimd.memset(masks[:, :], 1.0)
        for j in range(4):
            w = 512 - j * 128
            nc.gpsimd.affine_select(
                out=masks[:, ds(MOFF[j], w)],
                in_=masks[:, ds(MOFF[j], w)],
                compare_op=ALU.is_ge,
                fill=0.0,
                base=0,
                pattern=[[1, w]],
                channel_multiplier=-1,
            )

        with (
            tc.tile_pool(name="stage1", bufs=1) as s1,
            tc.tile_pool(name="probs", bufs=3) as probp,
            tc.tile_pool(name="small", bufs=4) as smallp,
            tc.tile_pool(name="outp", bufs=4) as outp,
            tc.tile_pool(name="ps1", bufs=2, space="PSUM") as ps1,
            tc.tile_pool(name="ps_sc", bufs=2, space="PSUM") as ps_sc,
            tc.tile_pool(name="ps_pv", bufs=2, space="PSUM") as ps_pv,
        ):
            # three issue queues (sync/scalar/gpsimd); per-queue order
            # matched to the kt-major hp0 ramp (x and w tiles arrive in
            # KT_ORDER): scalar: x0 wk0 x1 wk1 x2 wk2..7, gpsimd: x3 x4 x5
            # wv0..7, sync: wq0..7 x6 x7 then the cold tensors
            xts = [None] * 8
            wqs, wks, wvs = [None] * 8, [None] * 8, [None] * 8

            def load(eng, lst, idx, src, shape, nmtag, halves=False):
                t = s1.tile(shape, F16, tag=f"{nmtag}{idx}", name="t_in")
                if halves:
                    h = shape[1] // 2
                    r = src[idx * 128 : (idx + 1) * 128, :]
                    eng.dma_start(out=t[:, 0:h], in_=r[:, 0:h])
                    eng.dma_start(out=t[:, h:], in_=r[:, h:])
                else:
                    eng.dma_start(
                        out=t[:, :], in_=src[idx * 128 : (idx + 1) * 128, :]
                    )
                lst[idx] = t

            for kt in range(8):
                load(nc.sync, wqs, kt, wq_d, [128, GD], "wq")
            for kt in range(8):
                load(nc.scalar, wks, kt, wk_d, [128, GD], "wk")
            for i in (0, 1, 2, 3, 4, 5):
                load(nc.gpsimd, xts, i, xT_d, [128, S], "xt")
            for i in (6, 7):
                load(nc.sync, xts, i, xT_d, [128, S], "xt")
            for kt in range(8):
                load(nc.gpsimd, wvs, kt, wv_d, [128, GD], "wv")

            # lower-priority DMAs after the hot stage-1 inputs
            nc.sync.dma_start(out=bq_sb[:, :], in_=bq_d[:, :])
            nc.sync.dma_start(out=bk_sb[:, :], in_=bk_d[:, :])
            nc.sync.dma_start(out=bv_sb[:, :], in_=bv_d[:, :])

            def load_cold():
                # out-projection inputs, first read ~2/3 into the kernel:
                # keep their 1.5MB out of the bandwidth-bound startup window
                nc.sync.dma_start(out=ident[:, :], in_=id_d[:, :])
                for kt in range(4):
                    nc.sync.dma_start(
                        out=wp_sb[:, ds(kt * D, D)],
                        in_=wp_d[kt * 128 : (kt + 1) * 128, :],
                    )
                nc.sync.dma_start(out=bp_sb[:, :], in_=bp_d[:, :])

            # ---- qkv pieces ------------------------------------------------
            def qk_group(hp, which, n, pool=None, tag="ps"):
                w, w_bias, dst = (
                    (wqs, bq_sb, qTs[hp]) if which == "q" else (wks, bk_sb, kTs[hp])
                )
                ps = (pool or ps1).tile([128, 512], F32, tag=tag, name="ps")
                for i, kt in enumerate(range(8)):
                    nc.tensor.matmul(
                        ps[:, :],
                        w[kt][:, ds(hp * 128, 128)],
                        xts[kt][:, ds(n * 512, 512)],
                        start=(i == 0),
                        stop=(i == 7),
                    )
                nc.vector.tensor_scalar_add(
                    out=dst[n][:, :],
                    in0=ps[:, :],
                    scalar1=w_bias[:, hp : hp + 1],
                )

            def emit_vgrp(hp, g):
                # v rows for key tiles 4g..4g+3 of head pair hp
                for rl in range(4):
                    rt = 4 * g + rl
                    ps = ps1.tile([128, 512], F32, tag="ps", name="ps")
                    for i, kt in enumerate(range(8)):
                        nc.tensor.matmul(
                            ps[0:128, 0:128],
                            xts[kt][:, ds(rt * 128, 128)],
                            wvs[kt][:, ds(hp * 128, 128)],
                            start=(i == 0),
                            stop=(i == 7),
                        )
                    # interleaved store: local head hl -> cols
                    # [hl*65, hl*65+64), + bias
                    out_ap = vs[hp][g][:, ds(rl * 130, 130)].rearrange(
                        "p (h c) -> p h c", h=2
                    )[:, :, 0:64]
                    in_ap = ps[:, 0:128].rearrange("p (h c) -> p h c", h=2)
                    bv_ap = bv_sb[:, ds(hp * 128, 128)].rearrange(
                        "p (h c) -> p h c", h=2
                    )
                    nc.vector.tensor_add(out=out_ap, in0=in_ap, in1=bv_ap)
                # ones columns
                ones_ap = vs[hp][g][:, :].rearrange("p (r c) -> p r c", c=65)[
                    :, :, 64:65
                ]
                nc.gpsimd.memset(ones_ap, 1.0)

            # out-projection piece: one (row-tile, col-half) psum group;
            # the bias-add runs on gpsimd so the DVE queue stays clear for
            # the attention normalize chain
            def st3_piece(c3, rt, nch, pool=None, tag="ps", dma_eng=None):
                def f():
                    ps = (pool or ps1).tile([128, 512], F32, tag=tag, name="ps")
                    for kt4 in range(4):
                        nc.tensor.matmul(
                            ps[:, :],
                            aTc[c3][:, ds(kt4 * 512 + (rt % 4) * 128, 128)],
                            wp_sb[:, ds(kt4 * D + nch * 512, 512)],
                            start=(kt4 == 0),
                            stop=(kt4 == 3),
                        )
                    osb = outp.tile([128, 512], F16, tag="osb", name="osb")
                    nc.vector.tensor_add(
                        out=osb[:, :],
                        in0=ps[:, :],
                        in1=bp_sb[:, nch * 512 : (nch + 1) * 512],
                    )
                    (dma_eng or nc.sync).dma_start(
                        out=out_d[
                            rt * 128 : (rt + 1) * 128,
                            nch * 512 : (nch + 1) * 512,
                        ],
                        in_=osb[:, :],
                    )
                return f

            # two interleave queues: pending = pv/normalize/transpose pieces
            # of the previous chunk (critical path), fillers = qkv slices of
            # the next head pair / out-projection chunks (bulk PE work)
            pending = deque()
            fillers = deque()

            def fill_one(gi, hp):
                # first slot: drain the previous chunk's pv chain (its exp
                # deps are at the front of the in-order ACT queue, and its
                # transposes gate the out-projection). Later slots: bulk
                # qkv/out-proj fillers, double rate during hp3 so the
                # out-projection chunks never pile up in the tail.
                if gi == 1:
                    while pending:
                        pending.popleft()()
                elif fillers:
                    fillers.popleft()()
                    if hp == 3 and fillers:
                        fillers.popleft()()

            def emit_scores(hp, c):  # noqa: C901
                q0 = c * 512
                if c > 0:
                    prA = probp.tile([128, 12 * 512], BF16, tag="probs", name="prA")
                    prB = probp.tile([128, 12 * 512], BF16, tag="probs", name="prB")
                else:
                    prA = prB = None
                prDA = probp.tile(
                    [128, 4 * 512], BF16, tag="probsD", name="prDA", bufs=4
                )
                prDB = probp.tile(
                    [128, 4 * 512], BF16, tag="probsD", name="prDB", bufs=4
                )
                # diagonal pairs first: their exp+mask chains complete while
                # the clean exps run, so every pv group's final (diagonal)
                # accumulation step is ready in time
                g_order = [4 * c, 4 * c + 2] + list(range(0, 4 * c, 2))
                for gi, g in enumerate(g_order):
                    scA = ps_sc.tile([128, 1024], F32, tag="sc", name="scA")
                    scB = ps_sc.tile([128, 1024], F32, tag="sc", name="scB")
                    for j in (0, 1):
                        kt = g + j
                        # columns q < dd of diagonal tiles are fully masked:
                        # skip them in the matmul
                        dd = max(0, kt * 128 - q0)
                        kt_t = kTs[hp][kt // 4]
                        kcol = ds((kt % 4) * 128, 128)
                        nc.tensor.matmul(
                            scA[:, j * 512 + dd : (j + 1) * 512],
                            kt_t[0:64, kcol],
                            qTs[hp][c][0:64, ds(dd, 512 - dd)],
                            start=True, stop=True,
                        )
                        nc.tensor.matmul(
                            scB[:, j * 512 + dd : (j + 1) * 512],
                            kt_t[64:128, kcol],
                            qTs[hp][c][64:128, ds(dd, 512 - dd)],
                            start=True, stop=True,
                        )
                    if g >= 4 * c:
                        # diagonal tiles: exp the written column ranges; for
                        # the first pair the two ranges merge into one
                        # instruction across the 128-col stale gap (bounded
                        # stale scores, and the gap region of prD is never
                        # read) -- saves ACT per-instruction overhead in the
                        # exp-paced windows
                        gl = g - 4 * c
                        dd0 = gl * 128
                        dd1 = (gl + 1) * 128
                        for sc_t, pr_t in ((scA, prDA), (scB, prDB)):
                            if gl == 0:
                                nc.scalar.activation(
                                    out=pr_t[:, ds(0, 1024)],
                                    in_=sc_t[:, 0:1024], func=AF.Exp,
                                )
                                continue
                            nc.scalar.activation(
                                out=pr_t[:, ds(gl * 512 + dd0, 512 - dd0)],
                                in_=sc_t[:, dd0:512], func=AF.Exp,
                            )
                            nc.scalar.activation(
                                out=pr_t[:, ds((gl + 1) * 512 + dd1, 512 - dd1)],
                                in_=sc_t[:, 512 + dd1 : 1024], func=AF.Exp,
                            )
                        # causal mask on the two diagonal key tiles just
                        # exp'd: zero where key k0+i > query q0+j (bf16 2x)
                        for j2 in (gl, gl + 1):
                            dd = j2 * 128
                            for pr in (prDA, prDB):
                                nc.vector.tensor_mul(
                                    out=pr[:, ds(j2 * 512 + dd, 512 - dd)],
                                    in0=pr[:, ds(j2 * 512 + dd, 512 - dd)],
                                    in1=masks[:, ds(MOFF[j2], 512 - dd)],
                                )
                    else:
                        nc.scalar.activation(
                            out=prA[:, ds(g * 512, 1024)],
                            in_=scA[:, :], func=AF.Exp,
                        )
                        nc.scalar.activation(
                            out=prB[:, ds(g * 512, 1024)],
                            in_=scB[:, :], func=AF.Exp,
                        )
                    fill_one(gi, hp)
                return prA, prB, prDA, prDB

            # pv pieces for (hp, c): 8 accumulation chains + 2 reciprocals +
            # 4 normalize+transpose blocks, drained into later score windows
            def pv_pieces(hp, c, probs):
                prA, prB, prDA, prDB = probs
                state = {}

                def chain(hl, qq):
                    def f():
                        pr, prD = (prA, prDA) if hl == 0 else (prB, prDB)
                        if qq == 0:
                            state[hl] = ps_pv.tile(
                                [128, 260], F32, tag="pv", name="apv"
                            )
                        apv = state[hl]
                        qb = 4 * c + qq
                        for kt in range(qb + 1):
                            if kt < 4 * c:
                                lhsT = pr[:, ds(kt * 512 + qq * 128, 128)]
                            else:
                                j = kt - 4 * c
                                lhsT = prD[:, ds(j * 512 + qq * 128, 128)]
                            nc.tensor.matmul(
                                apv[:, ds(qq * 65, 65)],
                                lhsT,
                                vs[hp][kt // 4][:, ds((kt % 4) * 130 + hl * 65, 65)],
                                start=(kt == 0),
                                stop=(kt == qb),
                            )
                    return f

                def recip(hl, half):
                    def f():
                        apv = state[hl]
                        rec = smallp.tile([128, 2], F32, tag="rec", name="rec")
                        nc.vector.reciprocal(
                            out=rec.rearrange("p (q o) -> p q o", o=1),
                            in_=apv.rearrange("p (q v) -> p q v", v=65)[
                                :, 2 * half : 2 * half + 2, 64:65
                            ],
                        )
                        state[f"rec{hl}_{half}"] = rec
                    return f

                def norm_tp(qq):
                    def f():
                        amrg = smallp.tile([128, 128], F16, tag="amrg", name="amrg")
                        for hl in (0, 1):
                            nc.vector.tensor_scalar_mul(
                                out=amrg[:, ds(hl * 64, 64)],
                                in0=state[hl][:, ds(qq * 65, 64)],
                                scalar1=state[f"rec{hl}_{qq // 2}"][
                                    :, qq % 2 : qq % 2 + 1
                                ],
                            )
                        if hp == 3:
                            # PE transpose + DVE copy: ~4x lower latency
                            # than the DMA xbar path, and here the latency
                            # gates the out-projection
                            tp = ps_pv.tile([128, 128], F16, tag="pv", name="tp")
                            nc.tensor.matmul(
                                tp[:, :], amrg[:, :], ident[:, :],
                                is_transpose=True,
                            )
                            nc.vector.tensor_copy(
                                out=aTc[c][:, ds(hp * 512 + qq * 128, 128)],
                                in_=tp[:, :],
                            )
                        else:
                            # sync queue: SP has no compute duties, so the
                            # transpose's wait can't block exp decode
                            nc.sync.dma_start_transpose(
                                out=aTc[c][:, ds(hp * 512 + qq * 128, 128)],
                                in_=amrg[:, :],
                            )
                    return f

                out = []
                for half in (0, 1):
                    for hl in (0, 1):
                        out.append(chain(hl, 2 * half))
                        out.append(chain(hl, 2 * half + 1))
                    out.append(recip(0, half))
                    out.append(recip(1, half))
                    out.append(norm_tp(2 * half))
                    out.append(norm_tp(2 * half + 1))
                return out

            # ---- hp0 qkv ramp: kt-major across six borrowed psum groups so
            # PE consumes input tiles in DMA-arrival order
            dps = ps1.tile([128, 512], F32, tag="ps", name="ps")
            for _w in range(12):
                nc.tensor.matmul(
                    dps[:, :], dums[:, 0:128], dums[:, :],
                    start=True, stop=True,
                )
            # all eight hp0 q/k groups ride the ramp kt-major: ps1 and pv
            # slots hold one group each, each 2-bank sc slot holds two
            # (independent 512-col accumulation regions in one tile)
            w1_ps = [ps1.tile([128, 512], F32, tag="ps", name="ps") for _ in (0, 1)]
            sc_sh = [
                ps_sc.tile([128, 1024], F32, tag="sc", name="scsh") for _ in (0, 1)
            ]
            pv_sh = [
                ps_pv.tile([128, 512], F32, tag="pv", name="ps") for _ in (0, 1)
            ]
            wave1 = [
                ("q", 0, w1_ps[0][:, :]), ("q", 1, w1_ps[1][:, :]),
                ("q", 2, sc_sh[0][:, 0:512]), ("q", 3, sc_sh[0][:, 512:1024]),
                ("k", 0, sc_sh[1][:, 0:512]), ("k", 1, sc_sh[1][:, 512:1024]),
                ("k", 2, pv_sh[0][:, :]), ("k", 3, pv_sh[1][:, :]),
            ]
            for i, kt in enumerate(KT_ORDER):
                for which, n, ps in wave1:
                    w = wqs if which == "q" else wks
                    nc.tensor.matmul(
                        ps,
                        w[kt][:, ds(0, 128)],
                        xts[kt][:, ds(n * 512, 512)],
                        start=(i == 0),
                        stop=(i == 7),
                    )
            for which, n, ps in wave1:
                w_bias, dst = (bq_sb, qTs[0]) if which == "q" else (bk_sb, kTs[0])
                nc.vector.tensor_scalar_add(
                    out=dst[n][:, :], in0=ps, scalar1=w_bias[:, 0:1]
                )
            for g in range(2):
                emit_vgrp(0, g)

            for hp in range(4):
                if hp == 0:
                    for g in (2, 3):
                        fillers.append(lambda gg=g: emit_vgrp(0, gg))
                if hp == 1:
                    load_cold()
                if hp < 3:
                    nxt = hp + 1
                    for which in ("q", "k"):
                        # defer the n>=2 chunks of head-pair 3's projection
                        # into hp3's early filler slots: they aren't read
                        # until windows (3,2)/(3,3), and they keep the
                        # out-projection pieces from popping before their
                        # aTc transposes have landed
                        ns = (0, 1) if nxt == 3 else (0, 1, 2, 3)
                        for n in ns:
                            fillers.append(
                                (lambda w=which, nn=n: qk_group(nxt, w, nn))
                            )
                    for g in range(4):
                        fillers.append(lambda gg=g, h=nxt: emit_vgrp(h, gg))
                else:
                    for which in ("q", "k"):
                        for n in (2, 3):
                            fillers.append(
                                (lambda w=which, nn=n: qk_group(3, w, nn))
                            )
                for c in range(4):
                    probs = emit_scores(hp, c)
                    while pending:
                        pending.popleft()()
                    pending.extend(pv_pieces(hp, c, probs))
                    if hp == 3 and c >= 1:
                        for rt in range(4 * (c - 1), 4 * (c - 1) + 4):
                            for nch in range(2):
                                fillers.append(st3_piece(c - 1, rt, nch))
                while fillers:
                    fillers.popleft()()
            # tail: pending holds pv(3,3) as [half0 x8, half1 x8] with the
            # qq0/qq1 transposes at half0's end; run half1's chains while
            # the first transposes land, then the final out-projection
            # groups in qq order
            tailp = list(pending)
            pending.clear()
            for p in tailp[0:8]:
                p()
            for p in tailp[8:14]:
                p()
            st3_piece(3, 12, 0)()
            st3_piece(3, 12, 1)()
            st3_piece(3, 13, 0)()
            st3_piece(3, 13, 1)()
            for p in tailp[14:16]:
                p()
            st3_piece(3, 14, 0, ps_sc, "sc")()
            st3_piece(3, 14, 1, ps_sc, "sc", dma_eng=nc.scalar)()
            st3_piece(3, 15, 0, dma_eng=nc.scalar)()
            # final group: half-granularity add+DMA so the last transfer
            # starts as early as possible
            psf = ps1.tile([128, 512], F32, tag="ps", name="ps")
            for kt4 in range(4):
                nc.tensor.matmul(
                    psf[:, :],
                    aTc[3][:, ds(kt4 * 512 + 3 * 128, 128)],
                    wp_sb[:, ds(kt4 * D + 512, 512)],
                    start=(kt4 == 0),
                    stop=(kt4 == 3),
                )
            for hh in (0, 1):
                osbf = outp.tile([128, 256], F16, tag="osbh", name="osb")
                nc.vector.tensor_add(
                    out=osbf[:, :],
                    in0=psf[:, ds(hh * 256, 256)],
                    in1=bp_sb[:, 512 + hh * 256 : 512 + (hh + 1) * 256],
                )
                (nc.sync if hh == 0 else nc.scalar).dma_start(
                    out=out_d[15 * 128 : 16 * 128,
                              512 + hh * 256 : 512 + (hh + 1) * 256],
                    in_=osbf[:, :],
                )

    nc.compile()
    return nc


def make_in_maps(x, w_attn, b_attn, w_proj, b_proj):
    """Build the 8 per-core input maps (core 2b+g: batch b, heads 8g..8g+8)."""
    x = np.asarray(x, np.float32)
    w_attn = np.asarray(w_attn, np.float32)
    b_attn = np.asarray(b_attn, np.float32)
    w_proj = np.asarray(w_proj, np.float32)
    b_proj = np.asarray(b_proj, np.float32)

    in_maps = []
    for core in range(N_CORES):
        b, g = core // 2, core % 2
        c0 = g * GD
        wq = w_attn[:, c0 : c0 + GD]
        wk = w_attn[:, D + c0 : D + c0 + GD]
        wv = w_attn[:, 2 * D + c0 : 2 * D + c0 + GD]
        bq = b_attn[c0 : c0 + GD]
        bk = b_attn[D + c0 : D + c0 + GD]
        bv = b_attn[2 * D + c0 : 2 * D + c0 + GD]
        wp = w_proj[c0 : c0 + GD, :]
        in_maps.append(
            {
                "xT": np.ascontiguousarray(x[b].T).astype(np.float16),
                "wq": wq.astype(np.float16),
                "wk": wk.astype(np.float16),
                "wv": wv.astype(np.float16),
                "wp": wp.astype(np.float16),
                "bq": np.ascontiguousarray(bq.reshape(4, 128).T),
                "bk": np.ascontiguousarray(bk.reshape(4, 128).T),
                "bv": np.broadcast_to(bv, (128, GD)).copy(),
                "bp": np.broadcast_to(b_proj * 0.5, (128, D)).copy(),
                "ident": np.eye(128, dtype=np.float16),
            }
        )
    return in_maps


_CACHED_NC = None


def kernel(x, w_attn, b_attn, w_proj, b_proj, _trace=False):
    global _CACHED_NC
    from concourse.bass_utils import run_bass_kernel_spmd

    if _CACHED_NC is None:
        _CACHED_NC = build_bass()
    nc = _CACHED_NC

    in_maps = make_in_maps(x, w_attn, b_attn, w_proj, b_proj)
    res = run_bass_kernel_spmd(
        nc, in_maps, core_ids=list(range(N_CORES)), trace=_trace
    )
    outs = [r["out"] for r in res.results]
    B = np.asarray(x).shape[0]
    full = np.empty((B, S, D), np.float32)
    for b in range(B):
        full[b] = outs[2 * b].astype(np.float32) + outs[2 * b + 1].astype(
            np.float32
        )
    kernel.last_result = res
    return full
